# revision 13
# baseline (speedup 1.0000x reference)
"""Trainium2 Bass kernel for nn_DrugGraphEmbedding (2-layer GCN over drug graphs).

Strategy: data-parallel over the batch axis across 8 NeuronCores (4 graphs per
core), fp16 end-to-end on the wide paths (inputs are ~N(0,1): fp16 range is
never an issue and its 11-bit mantissa keeps rel-err ~1e-3 << 2e-2 gate).

Per core:
  packed stream: each graph gathers <=128 distinct substructure rows (of 256),
           so the host packs exactly those rows per graph into a 128-row
           block.  The device streams [128, 2 halves, 16 pathways, 512] fp16
           (4.2MB vs the naive 16.8MB fp32) with 1-2KB contiguous descriptor
           runs.  Vector engine accumulates the pathway sum in fp16 (2x DVE).
  graph-halved pipeline: graphs {0,1} finish first -> gather-by-matmul
           (packed one-hots pre-scaled with dis/P), W1 -> node-major fp16
           h1 block -> AllGather-a starts while graphs {2,3} still stream;
           AllGather-b follows.  The dense-adjacency aggregation's u-tiles
           are host-permuted to (half, core, graph) order so aggregation
           pass A runs as soon as AllGather-a lands, overlapping -b.
  aggregation: dense-adjacency matmuls in fp16 (1 cycle/row), hh-major so
           the x1s epilogue of h-half 0 overlaps h-half 1's matmuls.
  layer 2 chains through W2 in the transposed layout; its node-major table
           is AllGathered in two halves the same way; a dis-weighted pooling
           reduction produces the per-graph mean in fp32.
"""

import os
import numpy as np

# ---- problem constants (hardcoded per contest rules) ----
B, P, S, D = 32, 16, 256, 256
NG = 128
N = B * NG            # 4096 nodes
E = 65536
H = 256
M = 8                 # cores
GPC = B // M          # 4 graphs per core
NPC = GPC * NG        # 512 nodes per core
NT = N // 128         # 32 node tiles

_CACHE = {}


# --------------------------------------------------------------------------
# host-side preprocessing (sharding + index prep)
# --------------------------------------------------------------------------
def _host_prep(global_ids, edge_index):
    gid = np.asarray(global_ids).astype(np.int64)
    ei = np.asarray(edge_index).astype(np.int64)
    src = np.concatenate([ei[0], np.arange(N, dtype=np.int64)])
    dst = np.concatenate([ei[1], np.arange(N, dtype=np.int64)])
    deg = np.bincount(dst, minlength=N).astype(np.float32)
    dis = (1.0 / np.sqrt(deg)).astype(np.float32)

    # u-tile permutation: jt = half*16 + 2c + j  <->  ut = 4c + 2*half + j
    perm = np.empty(NT, dtype=np.int64)
    for half in range(2):
        for c in range(M):
            for j in range(2):
                perm[half * 16 + 2 * c + j] = 4 * c + 2 * half + j

    # packed rows per graph: the distinct gid values (<=128), padded
    packrows = np.zeros((B, 128), dtype=np.int64)
    for gg in range(B):
        u = np.unique(gid[gg])
        packrows[gg, :len(u)] = u
        packrows[gg, len(u):] = u[-1]  # keep sorted for searchsorted

    per_core = []
    for c in range(M):
        lo, hi = NPC * c, NPC * (c + 1)
        m = (dst >= lo) & (dst < hi)
        es, ed = src[m], dst[m] - lo
        # dense adjacency [src%128, (src//128, dst_local)], u-tiles permuted
        adj = np.zeros((128, NT, NPC), dtype=np.float32)
        np.add.at(adj, (es % 128, es // 128, ed), 1.0)
        adj = adj[:, perm, :]
        # packed one-hot, scaled by dis/P: [sp, (g, i)]
        ohnx = np.zeros((128, GPC * 128), dtype=np.float32)
        for g in range(GPC):
            gg = GPC * c + g
            gnodes = lo + 128 * g + np.arange(NG)
            sp = np.searchsorted(packrows[gg], gid[gg])
            ohnx[sp, g * 128 + np.arange(NG)] = dis[gnodes] / P
        dis_c = dis[lo:hi]
        disTr = np.ascontiguousarray(np.broadcast_to(dis_c, (128, NPC)))
        pwTr = np.ascontiguousarray(disTr / NG)
        per_core.append(dict(
            adj=np.ascontiguousarray(adj.reshape(128, NT * NPC)).astype(np.float16),
            ohnx=ohnx.astype(np.float16),
            disTr32=disTr, disTr16=disTr.astype(np.float16), pwTr=pwTr,
        ))
    return per_core, packrows


# --------------------------------------------------------------------------
# the Bass program (one SPMD program for all 8 cores)
# --------------------------------------------------------------------------
def _build_program():
    import concourse.bacc as bacc
    import concourse.tile as tile
    import concourse.mybir as mybir
    from concourse.bass import _add_dep_helper
    from concourse.masks import make_identity

    f32 = mybir.dt.float32
    f16 = mybir.dt.float16
    AF = mybir.ActivationFunctionType
    ADD = mybir.AluOpType.add
    MUL = mybir.AluOpType.mult

    nc = bacc.Bacc("TRN2", target_bir_lowering=False, debug=False, num_devices=M)

    # emb layout: [sp, half, p-pair-chunk, (p2 j d)]
    emb_t = nc.dram_tensor("emb", [128, 2, 8, 1024], f16, kind="ExternalInput")
    ohnx_t = nc.dram_tensor("ohnx", [128, GPC * 128], f16, kind="ExternalInput")
    w1_t = nc.dram_tensor("w1", [128, 512], f16, kind="ExternalInput")
    w2_t = nc.dram_tensor("w2", [128, 512], f16, kind="ExternalInput")
    adj_t = nc.dram_tensor("adj", [128, NT * NPC], f16, kind="ExternalInput")
    disTr32_t = nc.dram_tensor("disTr32", [128, NPC], f32, kind="ExternalInput")
    disTr16_t = nc.dram_tensor("disTr16", [128, NPC], f16, kind="ExternalInput")
    pwTr_t = nc.dram_tensor("pwTr", [128, NPC], f32, kind="ExternalInput")
    b1c_t = nc.dram_tensor("b1c", [128, 2], f32, kind="ExternalInput")
    b2c_t = nc.dram_tensor("b2c", [128, 2], f32, kind="ExternalInput")
    out_t = nc.dram_tensor("out", [GPC, H], f32, kind="ExternalOutput")

    cc1_in = nc.dram_tensor("cc1_in", [NPC, 256], f16, kind="Internal")
    cc2_in = nc.dram_tensor("cc2_in", [NPC, 256], f16, kind="Internal")
    tables = [
        [nc.dram_tensor(f"table{l}{h}", [M * 256, 256], f16, kind="Internal",
                        addr_space="Shared") for h in range(2)]
        for l in (1, 2)
    ]
    RG = [list(range(M))]

    with tile.TileContext(nc) as tc:
        with (
            tc.tile_pool(name="const", bufs=1) as cpool,
            tc.tile_pool(name="stream", bufs=8) as spool,
            tc.tile_pool(name="work", bufs=1) as wpool,
            tc.tile_pool(name="hst", bufs=1) as hpool,
            tc.tile_pool(name="psum", bufs=4, space="PSUM") as ppool,
            tc.tile_pool(name="psmall", bufs=2, space="PSUM") as pspool,
        ):
            rings = [nc.sync, nc.scalar, nc.gpsimd]

            # ---- constants in ----
            ohnx_sb = cpool.tile([128, GPC * 128], f16, name="ohnx_sb")
            w1_sb = cpool.tile([128, 512], f16, name="w1_sb")
            w2_sb = cpool.tile([128, 512], f16, name="w2_sb")
            adj_sb = cpool.tile([128, NT * NPC], f16, name="adj_sb")
            disTr32_sb = cpool.tile([128, NPC], f32, name="disTr32_sb")
            disTr16_sb = cpool.tile([128, NPC], f16, name="disTr16_sb")
            pwTr_sb = cpool.tile([128, NPC], f32, name="pwTr_sb")
            b1c_sb = cpool.tile([128, 2], f32, name="b1c_sb")
            b2c_sb = cpool.tile([128, 2], f32, name="b2c_sb")
            ident16 = cpool.tile([128, 128], f16, name="ident16")
            ident32 = cpool.tile([128, 128], f32, name="ident32")

            nc.sync.dma_start(ohnx_sb[:], ohnx_t[:])
            nc.sync.dma_start(w1_sb[:], w1_t[:])
            nc.sync.dma_start(w2_sb[:], w2_t[:])
            nc.scalar.dma_start(disTr32_sb[:], disTr32_t[:])
            nc.scalar.dma_start(disTr16_sb[:], disTr16_t[:])
            nc.scalar.dma_start(pwTr_sb[:], pwTr_t[:])
            nc.scalar.dma_start(b1c_sb[:], b1c_t[:])
            nc.scalar.dma_start(b2c_sb[:], b2c_t[:])
            make_identity(nc, ident16[:])
            make_identity(nc, ident32[:])

            # ---- packed pathway-sum stream, graph-halved ----
            # half a chunks lead on every ring; half b chunks follow
            accs = [cpool.tile([128, 512], f16, name=f"acc{h}") for h in range(2)]
            ring_last = [None, None, None]
            order = [(0, q) for q in range(8)] + [(1, q) for q in range(8)]
            for idx, (h, q) in enumerate(order):
                r = idx % 3
                pt = spool.tile([128, 1024], f16, name="pt")
                dma = rings[r].dma_start(pt[:], emb_t[:][:, h, q, :])
                if ring_last[r] is not None:
                    _add_dep_helper(dma.ins, ring_last[r].ins, sync=False)
                ring_last[r] = dma
                for s in range(2):
                    if q == 0 and s == 0:
                        nc.vector.tensor_copy(accs[h][:], pt[:, :512])
                    else:
                        nc.vector.tensor_tensor(
                            accs[h][:], accs[h][:], pt[:, 512 * s:512 * (s + 1)],
                            op=ADD)

            # adjacency load: after the stream, off the sync ring (which
            # carries the latency-critical cc_in writes)
            for q in range(4):
                r = 1 + q % 2
                a_dma = rings[r].dma_start(
                    adj_sb[:, 4096 * q:4096 * (q + 1)],
                    adj_t[:][:, 4096 * q:4096 * (q + 1)],
                )
                _add_dep_helper(a_dma.ins, ring_last[r].ins, sync=False)

            # ---- gather + W1 per graph; AllGather per half ----
            def h1_block(g):
                h, j = g // 2, g % 2
                nxT_sb = wpool.tile([128, 256], f16, name="nxT_sb", tag="nxT",
                                    bufs=2)
                for dh in range(2):
                    ps = ppool.tile([128, 128], f32, name="nxps", tag="mm")
                    nc.tensor.matmul(
                        ps[:],
                        lhsT=accs[h][:, 256 * j + 128 * dh:256 * j + 128 * (dh + 1)],
                        rhs=ohnx_sb[:, 128 * g:128 * (g + 1)],
                        start=True, stop=True,
                    )
                    nc.vector.tensor_copy(nxT_sb[:, 128 * dh:128 * (dh + 1)], ps[:])
                hps = ppool.tile([128, 256], f32, name="h1ps", tag="mm")
                for dh in range(2):
                    nc.tensor.matmul(
                        hps[:],
                        lhsT=nxT_sb[:, 128 * dh:128 * (dh + 1)],
                        rhs=w1_sb[:, 256 * dh:256 * (dh + 1)],
                        start=(dh == 0), stop=(dh == 1),
                    )
                h1_sb = wpool.tile([128, 256], f16, name="h1_sb", tag="h1sb",
                                   bufs=2)
                nc.vector.tensor_copy(h1_sb[:], hps[:])
                nc.sync.dma_start(cc1_in[128 * g:128 * (g + 1), :], h1_sb[:])

            for h in range(2):
                for j in range(2):
                    h1_block(2 * h + j)
                nc.gpsimd.collective_compute(
                    "AllGather", mybir.AluOpType.bypass, replica_groups=RG,
                    ins=[cc1_in[256 * h:256 * (h + 1), :].opt()],
                    outs=[tables[0][h][:].opt()],
                )

            def load_hstab(hstab, table_h, h):
                """four chunked DMAs per half; chunk q covers 2 jt blocks."""
                for q in range(4):
                    eng = rings[q % 2]
                    eng.dma_start(
                        hstab[:, (16 * h + 4 * q) * 256:(16 * h + 4 * (q + 1)) * 256
                              ].rearrange("p (t d) -> p t d", t=4),
                        table_h[:][512 * q:512 * (q + 1), :].rearrange(
                            "(t up) d -> up t d", up=128),
                    )

            def aggregate(layer, epilogue):
                """hh-major dense aggregation; epilogue(hh, agg_psum) emitted
                right after each hh chain so it overlaps the next chain."""
                hstab = hpool.tile([128, NT * 256], f16, name="hstab",
                                   tag="hstab", bufs=2)
                for h in range(2):
                    load_hstab(hstab, tables[layer][h], h)
                for hh in range(2):
                    agg = ppool.tile([128, 512], f32, name=f"agg{hh}", tag="mm")
                    for jt in range(NT):
                        nc.tensor.matmul(
                            agg[:],
                            lhsT=hstab[:, 256 * jt + 128 * hh:256 * jt + 128 * (hh + 1)],
                            rhs=adj_sb[:, NPC * jt:NPC * (jt + 1)],
                            start=(jt == 0), stop=(jt == NT - 1),
                        )
                    epilogue(hh, agg)

            # ---- layer 1: x1s = dis*relu(dis*agg + b1), fp16 ----
            x1s = [wpool.tile([128, 512], f16, name=f"x1s{hh}") for hh in range(2)]

            def epi1(hh, agg):
                x1t = wpool.tile([128, 512], f32, name="x1t", tag="x1t", bufs=2)
                nc.vector.tensor_tensor(x1t[:], agg[:], disTr32_sb[:], op=MUL)
                x1r = wpool.tile([128, 512], f16, name="x1r", tag="x1r", bufs=2)
                nc.scalar.activation(x1r[:], x1t[:], AF.Relu,
                                     bias=b1c_sb[:, hh:hh + 1])
                nc.vector.tensor_tensor(x1s[hh][:], x1r[:], disTr16_sb[:], op=MUL)

            aggregate(0, epi1)

            # ---- h2sT = W2^T @ x1s (transposed) ----
            h2s_sb = [wpool.tile([128, 512], f16, name=f"h2s{hh}") for hh in range(2)]
            for hh in range(2):
                ps = ppool.tile([128, 512], f32, name="h2ps", tag="mm")
                for h1h in range(2):
                    nc.tensor.matmul(
                        ps[:],
                        lhsT=w2_sb[:, h1h * 256 + 128 * hh:h1h * 256 + 128 * (hh + 1)],
                        rhs=x1s[h1h][:],
                        start=(h1h == 0), stop=(h1h == 1),
                    )
                nc.vector.tensor_copy(h2s_sb[hh][:], ps[:])
            # transpose to node-major and AllGather per half
            for h in range(2):
                for ib in (2 * h, 2 * h + 1):
                    hs_sb = wpool.tile([128, 256], f16, name="hs_sb", tag="hs",
                                       bufs=2)
                    for hh in range(2):
                        tp = pspool.tile([128, 128], f16, name="tp16", tag="tp",
                                         padded_shape=[128, 1024])
                        nc.tensor.transpose(
                            tp[:], h2s_sb[hh][:, 128 * ib:128 * (ib + 1)],
                            ident16[:],
                        )
                        nc.vector.tensor_copy(hs_sb[:, 128 * hh:128 * (hh + 1)],
                                              tp[:])
                    nc.sync.dma_start(cc2_in[128 * ib:128 * (ib + 1), :], hs_sb[:])
                nc.gpsimd.collective_compute(
                    "AllGather", mybir.AluOpType.bypass, replica_groups=RG,
                    ins=[cc2_in[256 * h:256 * (h + 1), :].opt()],
                    outs=[tables[1][h][:].opt()],
                )

            # ---- layer 2 aggregation + dis-weighted mean pool (fp32) ----
            out_sb = wpool.tile([GPC, 256], f32, name="out_sb")

            def epi2(hh, agg):
                pm = wpool.tile([128, 512], f32, name="pm", tag="pm", bufs=2)
                nc.vector.tensor_tensor(pm[:], agg[:], pwTr_sb[:], op=MUL)
                pr = wpool.tile([128, GPC], f32, name="pr", tag="pr", bufs=2)
                nc.vector.tensor_reduce(
                    pr[:], pm[:].rearrange("h (g v) -> h g v", g=GPC),
                    axis=mybir.AxisListType.X, op=ADD,
                )
                nc.vector.tensor_tensor(
                    pr[:], pr[:], b2c_sb[:, hh:hh + 1].to_broadcast([128, GPC]),
                    op=ADD,
                )
                tp = pspool.tile([GPC, 128], f32, name="ptp", tag="ptp",
                                 padded_shape=[GPC, 512])
                nc.tensor.transpose(tp[:], pr[:], ident32[:])
                nc.vector.tensor_copy(out_sb[:, 128 * hh:128 * (hh + 1)], tp[:])

            aggregate(1, epi2)
            nc.sync.dma_start(out_t[:], out_sb[:])

    nc.compile()
    return nc


def _get_program():
    if "nc" not in _CACHE:
        _CACHE["nc"] = _build_program()
    return _CACHE["nc"]


# --------------------------------------------------------------------------
# entry point
# --------------------------------------------------------------------------
def build_in_maps(drug_graph_embedding, global_ids, edge_index, W1, b1, W2, b2):
    emb = np.asarray(drug_graph_embedding, dtype=np.float32)
    W1 = np.asarray(W1, dtype=np.float32)
    W2 = np.asarray(W2, dtype=np.float32)
    b1 = np.asarray(b1, dtype=np.float32)
    b2 = np.asarray(b2, dtype=np.float32)

    prep, packrows = _host_prep(global_ids, edge_index)
    emb16 = emb.astype(np.float16)  # [B, P, S, D]
    w1h = np.ascontiguousarray(
        W1.reshape(2, 128, 256).transpose(1, 0, 2).reshape(128, 512)
    ).astype(np.float16)
    w2h = np.ascontiguousarray(
        W2.reshape(2, 128, 256).transpose(1, 0, 2).reshape(128, 512)
    ).astype(np.float16)
    b1c = np.ascontiguousarray(b1.reshape(2, 128).T)
    b2c = np.ascontiguousarray(b2.reshape(2, 128).T)

    in_maps = []
    for c in range(M):
        pc = prep[c]
        # packed stream: [sp, half, p, j, d] -> [128, 2, 8, (p2 j d)=1024]
        embc = np.empty((128, 2, P, 2, D), dtype=np.float16)
        for g in range(GPC):
            gg = GPC * c + g
            # [P, 128, D] -> [128, P, D]
            embc[:, g // 2, :, g % 2, :] = emb16[gg, :, packrows[gg], :]
        embc = embc.reshape(128, 2, 8, 1024)
        in_maps.append({
            "emb": np.ascontiguousarray(embc),
            "ohnx": pc["ohnx"],
            "w1": w1h, "w2": w2h,
            "adj": pc["adj"],
            "disTr32": pc["disTr32"], "disTr16": pc["disTr16"],
            "pwTr": pc["pwTr"],
            "b1c": b1c, "b2c": b2c,
        })
    return in_maps


def kernel(drug_graph_embedding, global_ids, edge_index, W1, b1, W2, b2):
    in_maps = build_in_maps(drug_graph_embedding, global_ids, edge_index,
                            W1, b1, W2, b2)
    nc = _get_program()

    if os.environ.get("BASS_KERNEL_SIM", "0") == "1":
        from concourse.bass_interp import MultiCoreSim
        sim = MultiCoreSim(nc, num_cores=M)
        for c in range(M):
            core = sim.cores[c]
            for k, v in in_maps[c].items():
                core.tensor(k)[:] = v
        sim.simulate(check_with_hw=False)
        outs = [np.array(sim.cores[c].tensor("out")) for c in range(M)]
    else:
        from concourse import bass_utils
        res = bass_utils.run_bass_kernel_spmd(
            nc, in_maps, core_ids=list(range(M)),
            trace=os.environ.get("BASS_KERNEL_TRACE", "0") == "1",
        )
        _CACHE["last_results"] = res
        outs = [res.results[c]["out"] for c in range(M)]

    return np.concatenate([o.reshape(GPC, H) for o in outs], axis=0)
